# revision 21
# baseline (speedup 1.0000x reference)
"""Trainium2 Bass kernel for nn_CrossAttFA (retrieval_knn).

Math (reference):
  q = W @ x1 (1x1 conv, per-view), k = W @ x2, v = x3
  Q = l2norm(unfold3x3(q) regrouped to [b, L, 1800]), K likewise
  attn = Q @ K^T  [b, L, L];  idx = argmax(attn, -1)
  out = fold3x3(gather rows of unfold(v) by idx)

Device formulation (per batch b): fold the horizontal patch shift dx into
channels: qp[(a,c,dx), u] = q[a,c, uy-1, x+dx-1] over a vertically padded
50x48 pixel grid (u = uy*48+x, uy in [0,50)).  Then with
  S[u, v] = sum_ch qp[ch, u] * kp[ch, v]           (600-dim contraction)
  attn[n, m] = sum_{dy in 0..2} S[n + 48*dy, m + 48*dy]
argmax_m attn[n,m]/||K_m|| equals the reference argmax (column scale rk[m]
preserved; row scale 1/||Q_n|| does not affect argmax).

Device/host split: the device computes S (bf16 matmuls, fp32 accumulate)
and the 3-term diagonal box-sum, and ships the box-summed (unscaled)
attention rows to the host in bf16.  The host applies the rk column
scale in fp32, takes top-8 per row, rescores those candidates exactly in
fp64 and picks the true argmax.  Rows whose top-8 spread is too small to
certify coverage (v1 - v8 < THETA, with device_err << THETA measured)
get a full exact row recompute.

Engine schedule per core (8 cores = 2 batches x 4 slabs of 576 rows):
  PE     8 HAM warm-up matmuls (lifts the K=4/8 power throttle early),
         then S tiles; tile 0 (and chunks 0-2 of tile 1, on the 3 spare
         PSUM banks) accumulate kc-outer so matmuls start as soon as the
         first K-chunk DMA lands and overlap the whole input load.
  Scalar all psum->sbuf bf16 casts + half the staging DMA issues.
  DVE    the two box-sum adds, split per column-half and interleaved so
         chained-op semaphore stalls hide (dep is 2 queue slots back).
         GpSimd tensor ops and SWDGE accum-DMAs are NOT used: the former
         share the DVE's SBUF port (measured 2-6x mutual slowdown), the
         latter cost 1-2us of Q7 descriptor work per issue.
"""
import sys

sys.path.insert(0, '/opt/trn_rl_repo')
import numpy as np
import ml_dtypes

B, C, AH, AW, H, W_ = 2, 64, 5, 5, 48, 48
A = AH * AW                  # 25 views
L = H * W_                   # 2304 pixels
CH = A * C // 8 * 3          # 600 channels (a, c_out=8, dx=3)
CO = 8                       # conv output channels
CHP = 640                    # padded to 5 K-chunks of 128
UR = 2400                    # padded u-grid rows (50 x 48)
NCORES = 8
SLAB = L // 4                # 576 attn rows per core
USLAB = SLAB + 96            # S rows needed per core (incl. +48,+96 halo)
NT = 480                     # matmul moving free dim (psum bank = 512 fp32)
THETA = 0.022                # top-8 spread flag threshold (abs, scaled units)
KLAST = 88                   # real channels in the 5th K-chunk (600 - 512)
HW = L // 2                  # 1152
TOPK = 8                     # host-side top-k per row

_PROG = None


def _build_program():
    import concourse.bass as bass
    import concourse.bacc as bacc
    import concourse.mybir as mybir
    from concourse.tile import TileContext

    nc = bacc.Bacc('TRN2', target_bir_lowering=False, debug=False,
                   num_devices=NCORES)
    qpT_in = nc.declare_dram_parameter("qpT", [5, 128, USLAB],
                                       mybir.dt.bfloat16, isOutput=False)
    kpT_in = nc.declare_dram_parameter("kpT", [5, 128, UR],
                                       mybir.dt.bfloat16, isOutput=False)
    att_out = nc.declare_dram_parameter("att", [128, 5, L],
                                        mybir.dt.bfloat16, isOutput=True)

    n_sp = (USLAB + 127) // 128          # 6 S-row tiles (last is 32 rows)
    sp_rows = [min(128, USLAB - 128 * t) for t in range(n_sp)]
    n_chunks = UR // NT                  # 5 column chunks per S tile

    with TileContext(nc) as tc:
        with tc.tile_pool(name="inp", bufs=1) as inp, \
             tc.tile_pool(name="sp", bufs=6) as spp, \
             tc.tile_pool(name="acc", bufs=4) as accp, \
             tc.tile_pool(name="last", bufs=1) as lastp, \
             tc.tile_pool(name="ps", bufs=1, space="PSUM") as psp:

            kp_t = [inp.tile([128, UR], mybir.dt.bfloat16, tag=f"kp{i}",
                             name=f"kp{i}") for i in range(5)]
            qp_t = [inp.tile([128, USLAB], mybir.dt.bfloat16, tag=f"qp{i}",
                             name=f"qp{i}") for i in range(5)]

            def ps_tile(j, w=NT):
                return psp.tile([128, w], mybir.dt.float32, tag=f"ps{j}",
                                name=f"ps{j}")

            # PE HAM warm-up: ~3us of full-width matmuls on constant data
            # before the first input chunk lands, so the K=4/8 power
            # throttle lifts before the real accumulation starts.
            wW = inp.tile([128, 128], mybir.dt.bfloat16, tag="wW")
            wX = inp.tile([128, NT], mybir.dt.bfloat16, tag="wX")
            nc.gpsimd.memset(wW[:], 1.0)
            nc.gpsimd.memset(wX[:], 1.0)
            psw = ps_tile(0)
            for _ in range(8):
                nc.tensor.matmul(psw[:128, :], wW[:], wX[:],
                                 start=True, stop=True)

            # Input DMAs on the SP queue, ordered by first use.
            for i in range(5):
                rows_k = KLAST if i == 4 else 128
                nc.sync.dma_start(kp_t[i][:rows_k, :], kpT_in[i][:rows_k, :])
                nc.sync.dma_start(qp_t[i][:rows_k, :], qpT_in[i][:rows_k, :])

            # one tag, bufs=6: six rotating buffers, all six S tiles live
            sp_tiles = [spp.tile([128, UR], mybir.dt.bfloat16,
                                 tag="sp", name=f"sp{t}")
                        for t in range(n_sp)]

            # --- Phase A: S tiles 0 (all 5 chunks) and 1 (chunks 0-2)
            # accumulate kc-outer on 8 PSUM banks, gated only by the
            # per-chunk input DMA arrivals.
            psA = [ps_tile(j) for j in range(5)]
            psB = [ps_tile(5 + j) for j in range(3)]
            for kc in range(5):
                kk = KLAST if kc == 4 else 128
                st, sp_ = (kc == 0), (kc == 4)
                for j in range(5):
                    c0 = NT * j
                    nc.tensor.matmul(psA[j][:128, :],
                                     qp_t[kc][:kk, 0:128],
                                     kp_t[kc][:kk, c0:c0 + NT],
                                     start=st, stop=sp_)
                for j in range(3):
                    c0 = NT * j
                    nc.tensor.matmul(psB[j][:128, :],
                                     qp_t[kc][:kk, 128:256],
                                     kp_t[kc][:kk, c0:c0 + NT],
                                     start=st, stop=sp_)
            for j in range(5):
                nc.scalar.copy(sp_tiles[0][:, NT * j:NT * (j + 1)],
                               psA[j][:128, :])
            for j in range(3):
                nc.scalar.copy(sp_tiles[1][:, NT * j:NT * (j + 1)],
                               psB[j][:128, :])
            # S tile 1 chunks 3,4: banks 0,1 are free after tile-0 copies.
            for j in (3, 4):
                c0 = NT * j
                ps = ps_tile(j - 3)
                for kc in range(5):
                    kk = KLAST if kc == 4 else 128
                    nc.tensor.matmul(ps[:128, :],
                                     qp_t[kc][:kk, 128:256],
                                     kp_t[kc][:kk, c0:c0 + NT],
                                     start=(kc == 0), stop=(kc == 4))
                nc.scalar.copy(sp_tiles[1][:, c0:c0 + NT], ps[:128, :])

            # Rotate chunks across all 8 PSUM banks so tile t+1's first
            # chunk never WAW-waits on tile t's same-bank copy.
            bank_ctr = [2]  # phase A ended on banks 0,1 (t1 j3,j4)

            def make_sp(t):
                # j-outer, kc-inner: each bank turns over after its 5
                # accumulation matmuls, so the Scalar copy of chunk j
                # overlaps the matmuls of chunk j+1.
                rows = sp_rows[t]
                u0 = 128 * t
                for j in range(n_chunks):
                    c0 = NT * j
                    ps = ps_tile(bank_ctr[0] % 8)
                    bank_ctr[0] += 1
                    for kc in range(5):
                        kk = KLAST if kc == 4 else 128
                        nc.tensor.matmul(ps[:rows, :],
                                         qp_t[kc][:kk, u0:u0 + rows],
                                         kp_t[kc][:kk, c0:c0 + NT],
                                         start=(kc == 0), stop=(kc == 4))
                    if j == 2:
                        nc.vector.tensor_copy(
                            sp_tiles[t][:rows, c0:c0 + NT], ps[:rows, :])
                    else:
                        nc.scalar.copy(sp_tiles[t][:rows, c0:c0 + NT],
                                       ps[:rows, :])

            def attn_tile(t):
                # attn rows 128t..128t+128 (slab-local).  s48 stages into
                # the accumulator via plain DMA (partition shift), s96
                # into its own buffer; the DVE then does the two box-sum
                # adds per column-half, interleaved so each op's
                # dependency sits 2 queue slots back (hides the ~0.5-1us
                # same-engine semaphore stalls).  The unscaled box-summed
                # tile DMAs straight to DRAM for the host top-k.
                sab = accp.tile([128, L], mybir.dt.bfloat16, tag="sab")
                s96 = accp.tile([128, L], mybir.dt.bfloat16, tag="s96")
                nc.sync.dma_start(sab[0:80, :],
                                  sp_tiles[t][48:128, 48:48 + L])
                nc.scalar.dma_start(sab[80:128, :],
                                    sp_tiles[t + 1][0:48, 48:48 + L])
                nc.scalar.dma_start(s96[0:32, :],
                                    sp_tiles[t][96:128, 96:96 + L])
                nc.sync.dma_start(s96[32:128, :],
                                  sp_tiles[t + 1][0:96, 96:96 + L])
                for lo, hi in ((0, HW), (HW, L)):
                    nc.vector.tensor_add(sab[:, lo:hi], sab[:, lo:hi],
                                         s96[:, lo:hi])
                for lo, hi in ((0, HW), (HW, L)):
                    nc.vector.tensor_add(sab[:, lo:hi], sab[:, lo:hi],
                                         sp_tiles[t][:, lo:hi])
                nc.gpsimd.dma_start(att_out[:, t, :], sab[:])

            def attn_tile_last():
                # The ragged 64-row last tile is repacked as [128, L/2]:
                # partitions 0:64 hold columns [0, L/2), partitions 64:128
                # hold columns [L/2, L).  s96h (the only piece needing the
                # final 32-row S tile) joins last so the post-S5 tail is
                # just two half-width adds and the output DMA.
                t0h = lastp.tile([128, HW], mybir.dt.bfloat16, tag="t0h")
                s48h = lastp.tile([128, HW], mybir.dt.bfloat16, tag="s48h")
                s96h = lastp.tile([128, HW], mybir.dt.bfloat16, tag="s96h")
                nc.sync.dma_start(t0h[0:64, :], sp_tiles[4][0:64, 0:HW])
                nc.sync.dma_start(t0h[64:128, :], sp_tiles[4][0:64, HW:L])
                nc.scalar.dma_start(s48h[0:64, :],
                                    sp_tiles[4][48:112, 48:48 + HW])
                nc.scalar.dma_start(s48h[64:128, :],
                                    sp_tiles[4][48:112, 48 + HW:48 + L])
                for lo, hi in ((0, HW // 2), (HW // 2, HW)):
                    nc.vector.tensor_add(t0h[:, lo:hi], t0h[:, lo:hi],
                                         s48h[:, lo:hi])
                nc.scalar.dma_start(s96h[0:32, :],
                                    sp_tiles[4][96:128, 96:96 + HW])
                nc.scalar.dma_start(s96h[64:96, :],
                                    sp_tiles[4][96:128, 96 + HW:96 + L])
                nc.sync.dma_start(s96h[32:64, :],
                                  sp_tiles[5][0:32, 96:96 + HW])
                nc.sync.dma_start(s96h[96:128, :],
                                  sp_tiles[5][0:32, 96 + HW:96 + L])
                for lo, hi in ((0, HW // 2), (HW // 2, HW)):
                    nc.vector.tensor_add(t0h[:, lo:hi], t0h[:, lo:hi],
                                         s96h[:, lo:hi])
                nc.gpsimd.dma_start(att_out[:, 4, 0:HW], t0h[:])

            attn_tile(0)
            make_sp(2)
            attn_tile(1)
            make_sp(3)
            attn_tile(2)
            make_sp(4)
            attn_tile(3)
            make_sp(5)
            attn_tile_last()

    nc.compile()
    return nc


def _host_prep(x1, x2, w):
    """Build qp/kp [B,600,UR] fp32, their padded bf16 device forms,
    and rk64 [B,L]."""
    x1f = x1.transpose(0, 2, 3, 1, 4, 5).reshape(B, A, C, H, W_)
    x2f = x2.transpose(0, 2, 3, 1, 4, 5).reshape(B, A, C, H, W_)
    q = np.einsum('oc,bachw->baohw', w, x1f)   # [B, A, 8, H, W]
    k = np.einsum('oc,bachw->baohw', w, x2f)

    def chanshift(g):
        # g [B, A, 8, H, W] -> [B, 600, 50*48] with (a, c, dx) channels on a
        # vertically padded 50x48 grid
        gp = np.pad(g, ((0, 0), (0, 0), (0, 0), (0, 0), (1, 1)))
        sh = np.stack([gp[..., dx:dx + W_] for dx in range(3)], axis=3)
        sh = sh.reshape(B, CH, H, W_)
        sh = np.pad(sh, ((0, 0), (0, 0), (1, 1), (0, 0)))
        return np.ascontiguousarray(sh.reshape(B, CH, UR), dtype=np.float32)

    qp = chanshift(q)
    kp = chanshift(k)
    # rk[m] = 1 / ||K_m||, from padded per-pixel energy box-sums (fp64)
    ek = (k.astype(np.float64) ** 2).sum(axis=(1, 2))        # [B, H, W]
    ekp = np.pad(ek, ((0, 0), (1, 1), (1, 1)))
    kn = sum(ekp[:, dy:dy + H, dx:dx + W_]
             for dy in range(3) for dx in range(3))
    rk64 = (1.0 / np.maximum(np.sqrt(kn), 1e-12)).reshape(B, L)

    def to_dev(g):
        gb = g.astype(ml_dtypes.bfloat16)
        pad = np.zeros((B, CHP - CH, UR), ml_dtypes.bfloat16)
        return np.concatenate([gb, pad], axis=1).reshape(B, 5, 128, UR)

    return qp, kp, to_dev(qp), to_dev(kp), rk64


def _resolve_idx(qp, kp, rk64, cand, flags):
    """Pick the exact (fp64) argmax among device candidates; rows with
    uncertifiably small top-8 spread get a full-row recompute."""
    nc_ = cand.shape[-1]
    idx = np.zeros((B, L), np.int64)
    for b in range(B):
        cb = cand[b]                             # [L, nc_]
        q64 = qp[b].astype(np.float64)           # [600, UR]
        k64 = kp[b].astype(np.float64)
        score = np.zeros((L, nc_))
        for dy in (0, 48, 96):
            Qd = q64[:, dy:dy + L]               # [600, L]
            for c0 in range(0, L, 384):
                sl = slice(c0, c0 + 384)
                Kd = k64[:, cb[sl] + dy]         # [600, chunk, nc_]
                # batched (1 x c) @ (c x k) per row
                score[sl] += np.matmul(
                    Qd[:, sl].T[:, None, :], Kd.transpose(1, 0, 2))[:, 0, :]
        score *= rk64[b][cb]
        pick = np.argmax(score, axis=1)
        idx[b] = cb[np.arange(L), pick]

        flagged = np.where(flags[b])[0]
        if len(flagged):
            Qr = np.stack([q64[:, flagged + dy] for dy in (0, 48, 96)])
            Sr = np.einsum('dcr,cv->drv', Qr, k64)   # [3, R, UR]
            accs = (Sr[0][:, 0:L] + Sr[1][:, 48:48 + L]
                    + Sr[2][:, 96:96 + L]) * rk64[b][None, :]
            idx[b][flagged] = np.argmax(accs, axis=1)
    return idx


def _gather_fold(x3, idx):
    """Host epilogue: gather unfold(v) rows by idx and fold back."""
    v = x3.transpose(0, 2, 3, 1, 4, 5).reshape(B * A, C, H, W_)
    vp = np.pad(v, ((0, 0), (0, 0), (1, 1), (1, 1)))
    cols = np.stack([vp[:, :, i:i + H, j:j + W_]
                     for i in range(3) for j in range(3)], axis=2)
    V = cols.reshape(B, A, C * 9, L).transpose(0, 3, 1, 2).reshape(B, L, -1)
    outc = np.take_along_axis(V, idx[:, :, None], axis=1)
    p_v = C * 9
    outc = outc.reshape(B, L, A, p_v).transpose(0, 2, 3, 1)
    outc = outc.reshape(B * A, C, 3, 3, H, W_)
    out = np.zeros((B * A, C, H + 2, W_ + 2), np.float32)
    for i in range(3):
        for j in range(3):
            out[:, :, i:i + H, j:j + W_] += outc[:, :, i, j]
    out = out[:, :, 1:1 + H, 1:1 + W_]
    return np.ascontiguousarray(
        out.reshape(B, AH, AW, C, H, W_).transpose(0, 3, 1, 2, 4, 5))


def _make_in_maps(qpb, kpb):
    in_maps = []
    for core in range(NCORES):
        b, r = core // 4, core % 4
        u0 = SLAB * r
        in_maps.append({
            "qpT": np.ascontiguousarray(qpb[b][:, :, u0:u0 + USLAB]),
            "kpT": kpb[b],
        })
    return in_maps


def _topk_desc(arr, k):
    """top-k values+indices per row of arr [..., n], descending."""
    part = np.argpartition(-arr, k - 1, axis=-1)[..., :k]
    vals = np.take_along_axis(arr, part, axis=-1)
    order = np.argsort(-vals, axis=-1)
    return (np.take_along_axis(vals, order, axis=-1),
            np.take_along_axis(part, order, axis=-1))


def kernel(x1, x2, x3, W):
    global _PROG
    sys.path.insert(0, '/opt/trn_rl_repo')
    from concourse.bass_utils import run_bass_kernel_spmd

    x1 = np.asarray(x1, dtype=np.float32)
    x2 = np.asarray(x2, dtype=np.float32)
    x3 = np.asarray(x3, dtype=np.float32)
    w = np.asarray(W, dtype=np.float32)

    qp, kp, qpb, kpb, rk64 = _host_prep(x1, x2, w)
    in_maps = _make_in_maps(qpb, kpb)

    if _PROG is None:
        _PROG = _build_program()
    res = run_bass_kernel_spmd(_PROG, in_maps, list(range(NCORES)))

    cand = np.zeros((B, L, TOPK), np.int64)
    flags = np.zeros((B, L), bool)
    for core in range(NCORES):
        b, r = core // 4, core % 4
        base = SLAB * r
        av = res.results[core]["att"].astype(np.float32)    # [128, 5, L]
        rows = np.empty((SLAB, L), np.float32)
        rows[0:512] = av[:, 0:4, :].transpose(1, 0, 2).reshape(512, L)
        # repacked last tile: partitions p<64 hold columns [0, L/2) of
        # row 512+p, partitions p>=64 the upper half
        rows[512:SLAB, 0:HW] = av[0:64, 4, 0:HW]
        rows[512:SLAB, HW:L] = av[64:128, 4, 0:HW]
        rows *= rk64[b][None, :]
        mv, mi = _topk_desc(rows, TOPK)
        cand[b, base:base + SLAB] = mi
        flags[b, base:base + SLAB] = (mv[:, 0] - mv[:, TOPK - 1]) < THETA

    idx = _resolve_idx(qp, kp, rk64, cand, flags)
    return _gather_fold(x3, idx)
